# revision 2
# baseline (speedup 1.0000x reference)
"""Local 7x7-window per-channel attention (SASA-style) on 8 TRN2 NeuronCores.

V5 (from V3): reference math is q*(K_win + r), so the radd pass stays.
Deltas vs V3:
  - All S-folds AND EV-folds on PE identity matmuls (V3 ran 2 EV blocks +
    tail as DVE chains, ~14us of DVE). PE gets dense back-to-back work, so
    no warmers.
  - bf16 projections: weights + identity ship bf16 from host; X converts
    to bf16 per DMA band on the idle prologue DVE (f32 matmul = 4
    cycles/row, bf16 = 1).
  - V-proj emitted in the prologue ACT-idle window instead of between exp
    blocks.
  - Per-half endgame: recip(S_h) as soon as that half's S accumulation
    stops; final mul reads E PSUM directly.

Sharding: pure data-parallel. Core c owns image b=c//2, output-row half
h=c%2 (28 rows), split into two 14-row quarters stacked on partitions:
partition p = quarter*64 + channel. No collectives.
"""

import os
import sys

if "/opt/trn_rl_repo" not in sys.path:
    sys.path.insert(0, "/opt/trn_rl_repo")

import numpy as np

import concourse.bass as bass
import concourse.bacc as bacc
import concourse.tile as tile
from concourse import mybir
from concourse.bass_utils import run_bass_kernel_spmd

N_CORES = 8
KS = 7
PAD = 3
G = 8
DD = 8
C = 64
H = W = 56
B = 4
QR = 14
PR = QR + 2 * PAD   # 20
PW = W + 2 * PAD    # 62
P = QR * W          # 784
HP = P // 2         # 392

F32 = mybir.dt.float32
BF16 = mybir.dt.bfloat16
ALU = mybir.AluOpType
ACTF = mybir.ActivationFunctionType

# knobs
RADD_ASSIGN = os.environ.get("RADD_ASSIGN", "DDDDDDD" + "AAADDDD" * 6)
EVPE = [int(x) for x in os.environ.get("EVPE", "0,1,2,3,4,5,6").split(",")
        if x != ""]


def build_nc():
    nc = bacc.Bacc("TRN2", target_bir_lowering=False, debug=False,
                   num_devices=N_CORES)
    x_ap = nc.dram_tensor("x", [128, PR, PW], F32, kind="ExternalInput").ap()
    wq_ap = nc.dram_tensor("wq", [128, 128], BF16, kind="ExternalInput").ap()
    wk_ap = nc.dram_tensor("wk", [128, 128], BF16, kind="ExternalInput").ap()
    wv_ap = nc.dram_tensor("wv", [128, 128], BF16, kind="ExternalInput").ap()
    ib_ap = nc.dram_tensor("ib", [128, 128], BF16, kind="ExternalInput").ap()
    cst_ap = nc.dram_tensor("cst", [128, 2 + KS * KS], F32,
                            kind="ExternalInput").ap()
    out_ap = nc.dram_tensor("out", [128, QR, W], F32, kind="ExternalOutput").ap()

    with tile.TileContext(nc) as tc:
        with tc.tile_pool(name="const", bufs=1) as constp, \
             tc.tile_pool(name="planes", bufs=1) as planesp, \
             tc.tile_pool(name="big", bufs=1) as bigp, \
             tc.tile_pool(name="small", bufs=2) as smallp, \
             tc.tile_pool(name="psum", bufs=2, space="PSUM") as psump, \
             tc.tile_pool(name="psacc", bufs=1, space="PSUM") as psaccp:

            X = planesp.tile([128, PR, PW], F32)
            XB = planesp.tile([128, PR, PW], BF16)
            Wk = constp.tile([128, 128], BF16)
            nc.scalar.dma_start(out=Wk[:], in_=wk_ap[:])
            # row bands matched to K-proj chunks so matmuls start early
            nc.sync.dma_start(out=X[:, 0:5], in_=x_ap[:, 0:5])
            nc.sync.dma_start(out=X[:, 5:10], in_=x_ap[:, 5:10])
            nc.sync.dma_start(out=X[:, 10:15], in_=x_ap[:, 10:15])
            nc.sync.dma_start(out=X[:, 15:20], in_=x_ap[:, 15:20])
            CST = constp.tile([128, 2 + KS * KS], F32)
            Wq = constp.tile([128, 128], BF16)
            nc.scalar.dma_start(out=Wq[:], in_=wq_ap[:])
            nc.scalar.dma_start(out=CST[:], in_=cst_ap[:])
            Wv = constp.tile([128, 128], BF16)
            nc.scalar.dma_start(out=Wv[:], in_=wv_ap[:])
            IB = constp.tile([128, 128], BF16)
            nc.scalar.dma_start(out=IB[:], in_=ib_ap[:])
            BK = CST[:, 0:1]
            BV = CST[:, 1:2]
            RT = CST[:, 2:2 + KS * KS]
            # X -> bf16 per band (DVE is idle during the prologue)
            nc.vector.tensor_copy(XB[:, 0:5], X[:, 0:5])
            nc.vector.tensor_copy(XB[:, 5:10], X[:, 5:10])
            nc.vector.tensor_copy(XB[:, 10:15], X[:, 10:15])
            nc.vector.tensor_copy(XB[:, 15:20], X[:, 15:20])

            K = planesp.tile([128, PR, PW], BF16)
            V = planesp.tile([128, PR, PW], BF16)
            Q = planesp.tile([128, QR, W], BF16)

            Kflat = K[:].rearrange("p h w -> p (h w)")
            Vflat = V[:].rearrange("p h w -> p (h w)")
            Xbf = XB[:].rearrange("p h w -> p (h w)")
            step = 310

            def proj_chunk(dst, wmat, bias, j):
                ps = psump.tile([128, step], F32, tag="ps")
                nc.tensor.matmul(ps[:], wmat[:], Xbf[:, j:j + step],
                                 start=True, stop=True)
                if bias is None:
                    nc.scalar.copy(out=dst[:, j:j + step], in_=ps[:])
                else:
                    nc.scalar.add(out=dst[:, j:j + step], in_=ps[:],
                                  add=bias)

            def qproj_chunk(j):
                ps = psump.tile([128, KS * W], F32, tag="ps")
                nc.tensor.matmul(
                    ps[:], Wq[:],
                    XB[:, PAD + j * KS: PAD + (j + 1) * KS, PAD:PAD + W],
                    start=True, stop=True)
                nc.scalar.copy(
                    out=Q[:, j * KS:(j + 1) * KS, :].rearrange(
                        "p h w -> p (h w)"),
                    in_=ps[:])

            # K rows 0..13 first so block-0 radd/qmult can start early
            proj_chunk(Kflat, Wk, BK, 0)
            proj_chunk(Kflat, Wk, BK, 310)
            qproj_chunk(0)
            proj_chunk(Kflat, Wk, BK, 620)
            qproj_chunk(1)
            proj_chunk(Kflat, Wk, BK, 930)

            L = bigp.tile([128, KS * KS, P], BF16, tag="L")
            EV = bigp.tile([128, KS * KS, P], BF16, tag="EV")

            qf = Q[:].rearrange("p h w -> p (h w)")
            qb = bass.AP(tensor=qf.tensor, offset=qf.offset,
                         ap=[qf.ap[0], [0, KS], [1, P]])
            vap = V[:]

            # PSUM accumulators (each half = one bank)
            S0 = psaccp.tile([128, HP], F32, tag="S0")
            S1 = psaccp.tile([128, HP], F32, tag="S1")
            E0 = psaccp.tile([128, HP], F32, tag="E0")
            E1 = psaccp.tile([128, HP], F32, tag="E1")

            def radd(k):
                kh, kw = divmod(k, KS)
                kwin = K[:, kh:kh + QR, kw:kw + W]
                if RADD_ASSIGN[k] == "A":
                    nc.scalar.activation(out=L[:, k], in_=kwin,
                                         func=ACTF.Identity,
                                         bias=RT[:, k:k + 1])
                else:
                    nc.vector.tensor_scalar(
                        out=L[:, k], in0=kwin, scalar1=RT[:, k:k + 1],
                        scalar2=None, op0=ALU.add)

            def qmult(b):
                blk = L[:, b * KS:(b + 1) * KS]
                nc.vector.tensor_tensor(blk, blk, qb, ALU.mult)

            def expblk(b):
                blk = L[:, b * KS:(b + 1) * KS].rearrange("p k x -> p (k x)")
                nc.scalar.activation(out=blk, in_=blk, func=ACTF.Exp)

            def s_fold(b):
                for k in range(KS):
                    pl = L[:, b * KS + k]
                    st = (b == KS - 1 and k == KS - 1)
                    nc.tensor.matmul(S0[:], IB[:], pl[:, 0:HP],
                                     start=(b == 0 and k == 0), stop=st)
                    nc.tensor.matmul(S1[:], IB[:], pl[:, HP:P],
                                     start=(b == 0 and k == 0), stop=st)

            def evmult(b):
                vwin = bass.AP(
                    tensor=vap.tensor, offset=vap.offset + b * PW,
                    ap=[vap.ap[0], [1, KS], [PW, QR], [1, W]])
                eblk = L[:, b * KS:(b + 1) * KS].rearrange(
                    "p k (h w) -> p k h w", h=QR)
                oblk = EV[:, b * KS:(b + 1) * KS].rearrange(
                    "p k (h w) -> p k h w", h=QR)
                nc.vector.tensor_tensor(oblk, eblk, vwin, ALU.mult)

            evpe_sorted = sorted(set(EVPE) & set(range(KS)))
            chain_blocks = [b for b in range(KS) if b not in evpe_sorted]

            def ev_fold(b):
                if b in evpe_sorted:
                    for k in range(KS):
                        pl = EV[:, b * KS + k]
                        st = (b == evpe_sorted[-1] and k == KS - 1)
                        nc.tensor.matmul(
                            E0[:], IB[:], pl[:, 0:HP],
                            start=(b == evpe_sorted[0] and k == 0), stop=st)
                        nc.tensor.matmul(
                            E1[:], IB[:], pl[:, HP:P],
                            start=(b == evpe_sorted[0] and k == 0), stop=st)
                else:
                    c0 = chain_blocks[0]
                    if b != c0:
                        d = EV[:, c0 * KS:(c0 + 1) * KS]
                        s = EV[:, b * KS:(b + 1) * KS]
                        nc.vector.tensor_tensor(d, d, s, ALU.add)
                    if b == chain_blocks[-1]:
                        b0 = c0 * KS
                        T = EV
                        nc.vector.tensor_tensor(
                            T[:, b0:b0 + 3], T[:, b0:b0 + 3],
                            T[:, b0 + 3:b0 + 6], ALU.add)
                        nc.vector.tensor_tensor(
                            T[:, b0:b0 + 1], T[:, b0:b0 + 1],
                            T[:, b0 + 6:b0 + 7], ALU.add)
                        nc.vector.tensor_tensor(
                            T[:, b0:b0 + 1], T[:, b0:b0 + 1],
                            T[:, b0 + 1:b0 + 2], ALU.add)
                        nc.vector.tensor_tensor(
                            T[:, b0:b0 + 1], T[:, b0:b0 + 1],
                            T[:, b0 + 2:b0 + 3], ALU.add)

            def emit_radds(b):
                for kw in range(KS):
                    radd(b * KS + kw)

            # --- prologue of the pipeline ---
            emit_radds(0)
            qmult(0)
            # V proj rides the ACT-idle window while qmult(0) runs
            proj_chunk(Vflat, Wv, BV, 0)
            proj_chunk(Vflat, Wv, BV, 310)
            proj_chunk(Vflat, Wv, BV, 620)

            # --- software pipeline over kh-blocks ---
            # ACT slot: [radds-A(b+1), exp(b)]; DVE slot:
            # [radds-D(b+1), qmult(b+1), evmult(b-1)]; PE slot:
            # [S-fold(b), EV-fold(b-1)].
            for b in range(KS):
                if b + 1 < KS:
                    emit_radds(b + 1)
                    qmult(b + 1)
                expblk(b)
                s_fold(b)
                if b == 1:
                    proj_chunk(Vflat, Wv, BV, 930)
                if b >= 1:
                    evmult(b - 1)
                    ev_fold(b - 1)
            evmult(KS - 1)
            ev_fold(KS - 1)

            # --- endgame ---
            from concourse.dve_ops import (RECIPROCAL_APPROX_FAST,
                                           RECIP_APPROX_FAST_CONSTS)
            cs = RECIP_APPROX_FAST_CONSTS
            R = smallp.tile([128, P], F32, tag="R")
            OUTC = smallp.tile([128, P], F32, tag="OUTC")

            have_chain = len(chain_blocks) > 0
            for h in range(2):
                c0, c1 = h * HP, (h + 1) * HP
                Sh = (S0, S1)[h]
                Eh = (E0, E1)[h]
                nc.vector._custom_dve(
                    RECIPROCAL_APPROX_FAST, out=R[:, c0:c1], in0=Sh[:],
                    s0=cs["s0"], s1=cs["s1"], imm2=cs["imm2"])
                if have_chain:
                    cb0 = chain_blocks[0] * KS
                    nc.vector.tensor_tensor(
                        OUTC[:, c0:c1], Eh[:], EV[:, cb0, c0:c1], ALU.add)
                    nc.vector.tensor_mul(OUTC[:, c0:c1], OUTC[:, c0:c1],
                                         R[:, c0:c1])
                else:
                    nc.vector.tensor_mul(OUTC[:, c0:c1], Eh[:], R[:, c0:c1])
                nc.sync.dma_start(
                    out=out_ap[:, h * KS:(h + 1) * KS],
                    in_=OUTC[:, c0:c1].rearrange("p (h w) -> p h w", h=KS))

    nc.compile()
    return nc


def shard_inputs(x, wq, wk, bk, wv, bv, rel_x, rel_y):
    import ml_dtypes
    bf16 = ml_dtypes.bfloat16
    x_pad = np.zeros((B, C, H + 2 * PAD, W + 2 * PAD), dtype=np.float32)
    x_pad[:, :, PAD:PAD + H, PAD:PAD + W] = x

    def blockdiag(w):
        w64 = np.zeros((C, C), dtype=np.float32)
        for g in range(G):
            w64[g * DD:(g + 1) * DD, g * DD:(g + 1) * DD] = w[g].T
        w128 = np.zeros((128, 128), dtype=np.float32)
        w128[:64, :64] = w64
        w128[64:, 64:] = w64
        return w128.astype(bf16)

    wq128, wk128, wv128 = blockdiag(wq), blockdiag(wk), blockdiag(wv)
    bk128 = np.concatenate([bk, bk]).reshape(128, 1).astype(np.float32)
    bv128 = np.concatenate([bv, bv]).reshape(128, 1).astype(np.float32)

    rt64 = np.empty((C, KS, KS), dtype=np.float32)
    for g in range(G):
        for d in range(DD):
            if d < DD // 2:
                rt64[g * DD + d] = rel_x[d]
            else:
                rt64[g * DD + d] = rel_y[d - DD // 2]
    rt128 = np.concatenate([rt64, rt64]).reshape(128, KS * KS)
    rt128 = np.ascontiguousarray(rt128, dtype=np.float32)
    ident = np.eye(128, dtype=bf16)
    cst = np.concatenate([bk128, bv128, rt128], axis=1)
    cst = np.ascontiguousarray(cst, dtype=np.float32)

    in_maps = []
    for core in range(N_CORES):
        b, half = divmod(core, 2)
        r0 = half * 2 * QR
        xs = np.empty((128, PR, PW), dtype=np.float32)
        xs[:64] = x_pad[b, :, r0:r0 + PR, :]
        xs[64:] = x_pad[b, :, r0 + QR:r0 + QR + PR, :]
        in_maps.append({
            "x": xs, "wq": wq128, "wk": wk128, "wv": wv128, "cst": cst,
            "ib": ident,
        })
    return in_maps


def unshard_output(results):
    out = np.empty((B, C, H, W), dtype=np.float32)
    for core in range(N_CORES):
        b, half = divmod(core, 2)
        r0 = half * 2 * QR
        r = results[core]["out"]
        out[b, :, r0:r0 + QR, :] = r[:64]
        out[b, :, r0 + QR:r0 + 2 * QR, :] = r[64:]
    return out


_NC_CACHE = {}


def get_nc():
    if "nc" not in _NC_CACHE:
        _NC_CACHE["nc"] = build_nc()
    return _NC_CACHE["nc"]


def kernel(**inputs):
    nc = get_nc()
    in_maps = shard_inputs(**inputs)
    res = run_bass_kernel_spmd(nc, in_maps, core_ids=list(range(N_CORES)))
    return unshard_output(res.results)
